# revision 17
# baseline (speedup 1.0000x reference)
"""Trainium2 Bass kernel for per-sample Brownian-distance-covariance (BDC) pooling.

Problem: x [128, 640, 100] f32, t [1,1] f32 (log temperature).
  per sample: G = x @ x^T; dcov = d_i + d_j - 2G; dcov = max(dcov, 1e-4);
  z = sqrt(exp(t)*dcov + 1e-5); out = z - rowmean - colmean + totmean.
Output: [128, 409600] f32.

Strategy (8 NeuronCores, pure data parallel, 16 samples/core), v2:
  - Row-coalesced input layout: partition p holds dims 5p..5p+4, so the
    f32->bf16 cast load is one big-descriptor DMA per group of 2 samples.
  - d = ||x_i||^2 via Pool-engine square + DVE segmented reduce (keeps the
    Activation engine free for the sqrt pass, which is its roofline).
  - Gram via TensorE with the d_j rank-2 (bf16 hi/lo) row folded into the
    same PSUM accumulation; d_i enters via the per-partition activation
    bias, which also compensates bf16 rounding exactly on the diagonal.
  - hrow/mrow row-vectors are packed with ONE flat SBUF->SBUF DMA per group
    (t5/t10 [2NQ,128] flattens to [2, NQ*128] in natural order), packed in
    the free dim so all matmul operands start at partition 0.
  - Double centering: one scalar_tensor_tensor pass per chunk, split
    between DVE and Pool, reading the colmean broadcast directly from PSUM;
    its output AP de-permutes columns so each sample's result is a single
    contiguous [128, 3200] tile, stored with ONE output DMA per sample.
"""
import numpy as np
from contextlib import ExitStack

import concourse.bass as bass
import concourse.bacc as bacc
import concourse.tile as tile
from concourse import mybir
from concourse.bass_utils import run_bass_kernel_spmd

F32 = mybir.dt.float32
BF16 = mybir.dt.bfloat16
AF = mybir.ActivationFunctionType
OP = mybir.AluOpType

N_CORES = 8
B_TOTAL = 128
B_CORE = B_TOTAL // N_CORES  # 16
DIM = 640
M = 100
NCHUNK = DIM // 128  # 5
GSZ = 2
NG = B_CORE // GSZ  # 8 groups
NQ = NCHUNK * GSZ  # 10
# bank-safe col splits for the colmean matmul, per sample slot in mps2:
# slot bp starts at byte 2560*bp inside the PSUM tile, so bp=1 must split
# at the 4096-byte bank boundary (matmul out cannot cross a PSUM bank).
MPS_SPLITS = {0: (0, 512, 640), 1: (0, 384, 512, 640)}

_cached_nc = None


def build():
    nc = bacc.Bacc("TRN2", target_bir_lowering=False)
    x = nc.dram_tensor("x", [B_CORE, DIM, M], F32, kind="ExternalInput")
    consts = nc.dram_tensor("consts", [128, 2], F32, kind="ExternalInput")
    ident_in = nc.dram_tensor("ident", [128, 128], F32, kind="ExternalInput")
    out = nc.dram_tensor("out", [B_CORE, DIM * DIM], F32, kind="ExternalOutput")

    with tile.TileContext(nc) as tc, ExitStack() as ctx:
        const_p = ctx.enter_context(tc.tile_pool(name="const", bufs=1))
        xbp = ctx.enter_context(tc.tile_pool(name="xbp", bufs=3))
        sqp = ctx.enter_context(tc.tile_pool(name="sqp", bufs=2))
        gp = ctx.enter_context(tc.tile_pool(name="gp", bufs=3))
        xtp = ctx.enter_context(tc.tile_pool(name="xtp", bufs=5))
        zp = ctx.enter_context(tc.tile_pool(name="zp", bufs=6))
        opool = ctx.enter_context(tc.tile_pool(name="op", bufs=3))
        pk = ctx.enter_context(tc.tile_pool(name="pk", bufs=3))
        psamp = ctx.enter_context(tc.tile_pool(name="psamp", bufs=6))
        ps_g = ctx.enter_context(tc.tile_pool(name="psg", bufs=2, space="PSUM"))
        ps_m = ctx.enter_context(tc.tile_pool(name="psm", bufs=1, space="PSUM"))
        ps_x = ctx.enter_context(tc.tile_pool(name="psx", bufs=1, space="PSUM"))

        # ---- constants ----
        c_consts = const_p.tile([128, 2], F32)
        nc.sync.dma_start(c_consts[:], consts[:])
        neg2alpha = c_consts[:, 0:1]
        twoalpha = c_consts[:, 1:2]

        c_ident = const_p.tile([128, 128], BF16)
        nc.gpsimd.dma_start(c_ident[:], ident_in[:])

        c_ones2 = const_p.tile([2, 128], BF16)
        nc.vector.memset(c_ones2[:], 1.0)
        c_ones128 = const_p.tile([128, 128], F32)
        nc.vector.memset(c_ones128[:], 1.0)

        def in_dma(g):
            b0 = GSZ * g
            xbg = xbp.tile([128, GSZ, NCHUNK, M], BF16, tag="xb")
            nc.gpsimd.dma_start(
                xbg[:],
                x[b0 : b0 + GSZ].rearrange("s (p r) m -> p s r m", p=128),
            )
            return xbg

        def emit_head(g, xbg):
            # squares + segmented reduce -> d  [128, NQ]
            sqg = sqp.tile([128, GSZ, NCHUNK, M], F32, tag="sq")
            nc.gpsimd.tensor_mul(sqg[:], xbg[:], xbg[:])
            d_g = gp.tile([128, NQ], F32, tag="d")
            nc.vector.tensor_reduce(
                d_g[:], sqg[:], axis=mybir.AxisListType.X, op=OP.add
            )
            # per-sample transposes -> xT [100, 640]
            xTs = []
            for bp in range(GSZ):
                xps = ps_x.tile([M, DIM], BF16, tag="xps")
                for r in range(NCHUNK):
                    nc.tensor.transpose(
                        xps[:, r * 128 : (r + 1) * 128], xbg[:, bp, r, :], c_ident[:]
                    )
                xT = xtp.tile([M, DIM], BF16, tag="xT")
                nc.vector.tensor_copy(xT[:], xps[:])
                xTs.append(xT)
            # hi/lo split of -0.5*d, bias
            hstack = gp.tile([128, 2 * NQ], BF16, tag="hstack")
            nc.vector.tensor_scalar(
                out=hstack[:, 0:NQ], in0=d_g[:], scalar1=-0.5, scalar2=None,
                op0=OP.mult,
            )
            hres = gp.tile([128, NQ], F32, tag="hres")
            nc.vector.tensor_scalar(
                out=hres[:], in0=d_g[:], scalar1=-0.5, scalar2=None, op0=OP.mult
            )
            nc.vector.tensor_sub(hres[:], hres[:], hstack[:, 0:NQ])
            nc.vector.tensor_copy(hstack[:, NQ : 2 * NQ], hres[:])
            tmpb = gp.tile([128, NQ], F32, tag="tmpb")
            nc.vector.tensor_add(tmpb[:], d_g[:], hstack[:, 0:NQ])
            nc.vector.tensor_add(tmpb[:], tmpb[:], hstack[:, NQ : 2 * NQ])
            bias_g = gp.tile([128, NQ], F32, tag="bias")
            nc.vector.tensor_scalar(
                out=bias_g[:], in0=tmpb[:], scalar1=twoalpha, scalar2=1e-5,
                op0=OP.mult, op1=OP.add,
            )
            # transpose hi/lo stack and pack the per-sample [2, 640] rows
            xps2 = ps_x.tile([M, DIM], BF16, tag="xps")
            nc.tensor.transpose(xps2[0 : 2 * NQ, 0:128], hstack[:], c_ident[:])
            t5 = gp.tile([2 * NQ, 128], BF16, tag="t5")
            nc.vector.tensor_copy(t5[:], xps2[0 : 2 * NQ, 0:128])
            hrow2 = pk.tile([2, GSZ * DIM], BF16, tag="hrow")
            nc.sync.dma_start(hrow2[:], t5[:])
            rowsum_g = gp.tile([128, NQ], F32, tag="rowsum")
            return xTs, bias_g, hrow2, rowsum_g

        def tail_prep(pend):
            _g, _zs, rs_tots, rowsum_g = pend
            rm_g = gp.tile([128, NQ], F32, tag="rm")
            nc.vector.tensor_scalar(
                out=rm_g[:], in0=rowsum_g[:], scalar1=1.0 / DIM, scalar2=None,
                op0=OP.mult,
            )
            rmstack = gp.tile([128, 2 * NQ], BF16, tag="rmstack")
            nc.vector.tensor_copy(rmstack[:, 0:NQ], rm_g[:])
            rml = gp.tile([128, NQ], F32, tag="rml")
            nc.vector.tensor_sub(rml[:], rm_g[:], rmstack[:, 0:NQ])
            nc.vector.tensor_copy(rmstack[:, NQ : 2 * NQ], rml[:])
            xps3 = ps_x.tile([M, DIM], BF16, tag="xps")
            nc.tensor.transpose(xps3[0 : 2 * NQ, 0:128], rmstack[:], c_ident[:])
            t10 = gp.tile([2 * NQ, 128], BF16, tag="t10")
            nc.vector.tensor_copy(t10[:], xps3[0 : 2 * NQ, 0:128])
            mrow2 = pk.tile([2, GSZ * DIM], BF16, tag="mrow")
            nc.sync.dma_start(mrow2[:], t10[:])
            mps2 = ps_m.tile([128, GSZ, DIM], F32, tag="mps")
            s0s = []
            for bp in range(GSZ):
                nc.tensor.matmul(
                    mps2[:, bp, 0:1], c_ones128[:], rs_tots[bp][:],
                    start=True, stop=True, skip_group_check=True,
                )
                tm_b = psamp.tile([128, 1], F32, tag="tm")
                nc.scalar.mul(tm_b[:], mps2[:, bp, 0:1], 1.0 / (DIM * DIM))
                s0_b = psamp.tile([128, NCHUNK], F32, tag="s0")
                nc.vector.tensor_scalar(
                    out=s0_b[:], in0=rm_g[:, NCHUNK * bp : NCHUNK * (bp + 1)],
                    scalar1=tm_b[:], scalar2=None, op0=OP.subtract,
                )
                cuts = MPS_SPLITS[bp]
                for c0, c1 in zip(cuts[:-1], cuts[1:]):
                    nc.tensor.matmul(
                        mps2[:, bp, c0:c1], c_ones2[:],
                        mrow2[:, bp * DIM + c0 : bp * DIM + c1],
                        start=True, stop=True, skip_group_check=True,
                    )
                s0s.append(s0_b)
            return mps2, s0s

        def tail_sample(pend, tailst, bp):
            g, zs, _rs_tots, _rowsum_g = pend
            mps2, s0s = tailst
            b = GSZ * g + bp
            z, s0_b = zs[bp], s0s[bp]
            outt = opool.tile([128, NCHUNK, DIM], F32, tag="outt")
            mv = mps2[:, bp, :].rearrange("p (a b) -> p a b", a=NCHUNK)
            for r in range(NCHUNK):
                zv = z[:, r, :].rearrange("p (a b) -> p a b", a=NCHUNK)
                ov = outt[:, r, :].rearrange("p (b f) -> p f b", f=NCHUNK)
                nc.vector.scalar_tensor_tensor(
                    ov, zv, s0_b[:, r : r + 1], mv,
                    op0=OP.subtract, op1=OP.subtract,
                )
            nc.sync.dma_start(
                out[b].rearrange("(p c) -> p c", p=128), outt[:]
            )

        def phase_c(g, head, pend, tailst):
            xTs, bias_g, hrow2, rowsum_g = head
            zs, rs_tots = [], []
            for bp in range(GSZ):
                xT = xTs[bp]
                z = zp.tile([128, NCHUNK, DIM], F32, tag="z")
                for r in range(NCHUNK):
                    lhsT = xT[:, r * 128 : (r + 1) * 128]
                    ps = ps_g.tile([128, DIM], F32, tag="gram")
                    nc.tensor.matmul(
                        ps[:, 0:512], lhsT, xT[:, 0:512],
                        start=True, stop=False, skip_group_check=True,
                    )
                    nc.tensor.matmul(
                        ps[:, 512:640], lhsT, xT[:, 512:640],
                        start=True, stop=False, skip_group_check=True,
                    )
                    nc.tensor.matmul(
                        ps[:, 0:512], c_ones2[:],
                        hrow2[:, bp * DIM : bp * DIM + 512],
                        start=False, stop=True, skip_group_check=True,
                    )
                    nc.tensor.matmul(
                        ps[:, 512:640], c_ones2[:],
                        hrow2[:, bp * DIM + 512 : (bp + 1) * DIM],
                        start=False, stop=True, skip_group_check=True,
                    )
                    nc.scalar.activation(
                        z[:, r, :], ps[:], AF.Sqrt,
                        bias=bias_g[:, NCHUNK * bp + r : NCHUNK * bp + r + 1],
                        scale=neg2alpha,
                        accum_out=rowsum_g[:, NCHUNK * bp + r : NCHUNK * bp + r + 1],
                    )
                zs.append(z)
                rs_tot = psamp.tile([128, 1], F32, tag="rs")
                nc.vector.tensor_reduce(
                    rs_tot[:], rowsum_g[:, NCHUNK * bp : NCHUNK * (bp + 1)],
                    axis=mybir.AxisListType.X, op=OP.add,
                )
                rs_tots.append(rs_tot)
                if pend is not None:
                    tail_sample(pend, tailst, bp)
            return zs, rs_tots

        xbgs = {0: in_dma(0), 1: in_dma(1)}
        pending = None
        for g in range(NG):
            if g + 2 < NG:
                xbgs[g + 2] = in_dma(g + 2)
            head = emit_head(g, xbgs.pop(g))
            tailst = tail_prep(pending) if pending is not None else None
            zs, rs_tots = phase_c(g, head, pending, tailst)
            pending = (g, zs, rs_tots, head[3])

        tailst = tail_prep(pending)
        for bp in range(GSZ):
            tail_sample(pending, tailst, bp)

    nc.compile()
    return nc


def _get_nc():
    global _cached_nc
    if _cached_nc is None:
        _cached_nc = build()
    return _cached_nc


def make_in_maps(x: np.ndarray, t: np.ndarray):
    alpha = float(np.exp(t.astype(np.float64))[0, 0])
    consts = np.zeros((128, 2), dtype=np.float32)
    consts[:, 0] = -2.0 * alpha
    consts[:, 1] = 2.0 * alpha
    ident = np.eye(128, dtype=np.float32)
    xs = x.reshape(N_CORES, B_CORE, DIM, M)
    return [
        {"x": np.ascontiguousarray(xs[c]), "consts": consts, "ident": ident}
        for c in range(N_CORES)
    ]


def kernel(x: np.ndarray, t: np.ndarray) -> np.ndarray:
    x = np.asarray(x, dtype=np.float32)
    t = np.asarray(t, dtype=np.float32)
    nc = _get_nc()
    res = run_bass_kernel_spmd(nc, make_in_maps(x, t), core_ids=list(range(N_CORES)))
    return np.concatenate([r["out"] for r in res.results], axis=0)


# revision 19
# speedup vs baseline: 1.0088x; 1.0088x over previous
"""Trainium2 Bass kernel for per-sample Brownian-distance-covariance (BDC) pooling.

Problem: x [128, 640, 100] f32, t [1,1] f32 (log temperature).
  per sample: G = x @ x^T; dcov = d_i + d_j - 2G; dcov = max(dcov, 1e-4);
  z = sqrt(exp(t)*dcov + 1e-5); out = z - rowmean - colmean + totmean.
Output: [128, 409600] f32.

Strategy (8 NeuronCores, pure data parallel, 16 samples/core), v2:
  - Row-coalesced input layout: partition p holds dims 5p..5p+4, so the
    f32->bf16 cast load is one big-descriptor DMA per group of 2 samples.
  - d = ||x_i||^2 via Pool-engine square + DVE segmented reduce (keeps the
    Activation engine free for the sqrt pass, which is its roofline).
  - Gram via TensorE with the d_j rank-2 (bf16 hi/lo) row folded into the
    same PSUM accumulation; d_i enters via the per-partition activation
    bias, which also compensates bf16 rounding exactly on the diagonal.
  - hrow/mrow row-vectors are packed with ONE flat SBUF->SBUF DMA per group
    (t5/t10 [2NQ,128] flattens to [2, NQ*128] in natural order), packed in
    the free dim so all matmul operands start at partition 0.
  - Double centering: one scalar_tensor_tensor pass per chunk, split
    between DVE and Pool, reading the colmean broadcast directly from PSUM;
    its output AP de-permutes columns so each sample's result is a single
    contiguous [128, 3200] tile, stored with ONE output DMA per sample.
"""
import numpy as np
from contextlib import ExitStack

import concourse.bass as bass
import concourse.bacc as bacc
import concourse.tile as tile
from concourse import mybir
from concourse.bass_utils import run_bass_kernel_spmd

F32 = mybir.dt.float32
BF16 = mybir.dt.bfloat16
AF = mybir.ActivationFunctionType
OP = mybir.AluOpType

N_CORES = 8
B_TOTAL = 128
B_CORE = B_TOTAL // N_CORES  # 16
DIM = 640
M = 100
NCHUNK = DIM // 128  # 5
GSZ = 2
NG = B_CORE // GSZ  # 8 groups
NQ = NCHUNK * GSZ  # 10
# bank-safe col splits for the colmean matmul, per sample slot in mps2:
# slot bp starts at byte 2560*bp inside the PSUM tile, so bp=1 must split
# at the 4096-byte bank boundary (matmul out cannot cross a PSUM bank).
MPS_SPLITS = {0: (0, 512, 640), 1: (0, 384, 512, 640)}

_cached_nc = None


def build():
    nc = bacc.Bacc("TRN2", target_bir_lowering=False)
    x = nc.dram_tensor("x", [B_CORE, DIM, M], F32, kind="ExternalInput")
    consts = nc.dram_tensor("consts", [128, 2], F32, kind="ExternalInput")
    ident_in = nc.dram_tensor("ident", [128, 128], F32, kind="ExternalInput")
    out = nc.dram_tensor("out", [B_CORE, DIM * DIM], F32, kind="ExternalOutput")

    with tile.TileContext(nc) as tc, ExitStack() as ctx:
        const_p = ctx.enter_context(tc.tile_pool(name="const", bufs=1))
        xbp = ctx.enter_context(tc.tile_pool(name="xbp", bufs=3))
        sqp = ctx.enter_context(tc.tile_pool(name="sqp", bufs=2))
        gp = ctx.enter_context(tc.tile_pool(name="gp", bufs=3))
        xtp = ctx.enter_context(tc.tile_pool(name="xtp", bufs=5))
        zp = ctx.enter_context(tc.tile_pool(name="zp", bufs=6))
        opool = ctx.enter_context(tc.tile_pool(name="op", bufs=3))
        pk = ctx.enter_context(tc.tile_pool(name="pk", bufs=3))
        psamp = ctx.enter_context(tc.tile_pool(name="psamp", bufs=6))
        ps_g = ctx.enter_context(tc.tile_pool(name="psg", bufs=2, space="PSUM"))
        ps_m = ctx.enter_context(tc.tile_pool(name="psm", bufs=1, space="PSUM"))
        ps_x = ctx.enter_context(tc.tile_pool(name="psx", bufs=1, space="PSUM"))

        # ---- constants ----
        c_consts = const_p.tile([128, 2], F32)
        nc.sync.dma_start(c_consts[:], consts[:])
        neg2alpha = c_consts[:, 0:1]
        twoalpha = c_consts[:, 1:2]

        c_ident = const_p.tile([128, 128], BF16)
        nc.gpsimd.dma_start(c_ident[:], ident_in[:])

        c_ones2 = const_p.tile([2, 128], BF16)
        nc.vector.memset(c_ones2[:], 1.0)
        c_ones128 = const_p.tile([128, 128], F32)
        nc.vector.memset(c_ones128[:], 1.0)

        def in_dma(g):
            b0 = GSZ * g
            xbg = xbp.tile([128, GSZ, NCHUNK, M], BF16, tag="xb")
            nc.gpsimd.dma_start(
                xbg[:],
                x[b0 : b0 + GSZ].rearrange("s (p r) m -> p s r m", p=128),
            )
            return xbg

        def emit_head(g, xbg):
            # squares + segmented reduce -> d  [128, NQ]
            sqg = sqp.tile([128, GSZ, NCHUNK, M], F32, tag="sq")
            nc.gpsimd.tensor_mul(sqg[:], xbg[:], xbg[:])
            d_g = gp.tile([128, NQ], F32, tag="d")
            nc.vector.tensor_reduce(
                d_g[:], sqg[:], axis=mybir.AxisListType.X, op=OP.add
            )
            # per-sample transposes -> xT [100, 640]
            xTs = []
            for bp in range(GSZ):
                xps = ps_x.tile([M, DIM], BF16, tag="xps")
                for r in range(NCHUNK):
                    nc.tensor.transpose(
                        xps[:, r * 128 : (r + 1) * 128], xbg[:, bp, r, :], c_ident[:]
                    )
                xT = xtp.tile([M, DIM], BF16, tag="xT")
                nc.vector.tensor_copy(xT[:], xps[:])
                xTs.append(xT)
            # hi/lo split of -0.5*d, bias
            hstack = gp.tile([128, 2 * NQ], BF16, tag="hstack")
            nc.vector.tensor_scalar(
                out=hstack[:, 0:NQ], in0=d_g[:], scalar1=-0.5, scalar2=None,
                op0=OP.mult,
            )
            hres = gp.tile([128, NQ], F32, tag="hres")
            nc.vector.tensor_scalar(
                out=hres[:], in0=d_g[:], scalar1=-0.5, scalar2=None, op0=OP.mult
            )
            nc.vector.tensor_sub(hres[:], hres[:], hstack[:, 0:NQ])
            nc.vector.tensor_copy(hstack[:, NQ : 2 * NQ], hres[:])
            tmpb = gp.tile([128, NQ], F32, tag="tmpb")
            nc.vector.tensor_add(tmpb[:], d_g[:], hstack[:, 0:NQ])
            nc.vector.tensor_add(tmpb[:], tmpb[:], hstack[:, NQ : 2 * NQ])
            bias_g = gp.tile([128, NQ], F32, tag="bias")
            nc.vector.tensor_scalar(
                out=bias_g[:], in0=tmpb[:], scalar1=twoalpha, scalar2=1e-5,
                op0=OP.mult, op1=OP.add,
            )
            # transpose hi/lo stack and pack the per-sample [2, 640] rows
            xps2 = ps_x.tile([M, DIM], BF16, tag="xps")
            nc.tensor.transpose(xps2[0 : 2 * NQ, 0:128], hstack[:], c_ident[:])
            t5 = gp.tile([2 * NQ, 128], BF16, tag="t5")
            nc.vector.tensor_copy(t5[:], xps2[0 : 2 * NQ, 0:128])
            hrow2 = pk.tile([2, GSZ * DIM], BF16, tag="hrow")
            nc.sync.dma_start(hrow2[:], t5[:])
            rowsum_g = gp.tile([128, NQ], F32, tag="rowsum")
            return xTs, bias_g, hrow2, rowsum_g

        def tail_prep(pend):
            _g, _zs, rs_tots, rowsum_g = pend
            rm_g = gp.tile([128, NQ], F32, tag="rm")
            nc.vector.tensor_scalar(
                out=rm_g[:], in0=rowsum_g[:], scalar1=1.0 / DIM, scalar2=None,
                op0=OP.mult,
            )
            rmstack = gp.tile([128, 2 * NQ], BF16, tag="rmstack")
            nc.vector.tensor_copy(rmstack[:, 0:NQ], rm_g[:])
            rml = gp.tile([128, NQ], F32, tag="rml")
            nc.vector.tensor_sub(rml[:], rm_g[:], rmstack[:, 0:NQ])
            nc.vector.tensor_copy(rmstack[:, NQ : 2 * NQ], rml[:])
            xps3 = ps_x.tile([M, DIM], BF16, tag="xps")
            nc.tensor.transpose(xps3[0 : 2 * NQ, 0:128], rmstack[:], c_ident[:])
            t10 = gp.tile([2 * NQ, 128], BF16, tag="t10")
            nc.vector.tensor_copy(t10[:], xps3[0 : 2 * NQ, 0:128])
            mrow2 = pk.tile([2, GSZ * DIM], BF16, tag="mrow")
            nc.sync.dma_start(mrow2[:], t10[:])
            mps2 = ps_m.tile([128, GSZ, DIM], F32, tag="mps")
            s0s = []
            for bp in range(GSZ):
                nc.tensor.matmul(
                    mps2[:, bp, 0:1], c_ones128[:], rs_tots[bp][:],
                    start=True, stop=True, skip_group_check=True,
                )
                tm_b = psamp.tile([128, 1], F32, tag="tm")
                nc.scalar.mul(tm_b[:], mps2[:, bp, 0:1], 1.0 / (DIM * DIM))
                s0_b = psamp.tile([128, NCHUNK], F32, tag="s0")
                nc.vector.tensor_scalar(
                    out=s0_b[:], in0=rm_g[:, NCHUNK * bp : NCHUNK * (bp + 1)],
                    scalar1=tm_b[:], scalar2=None, op0=OP.subtract,
                )
                s0s.append(s0_b)
            return mps2, mrow2, s0s

        def tail_colmean(tailst):
            mps2, mrow2, _s0s = tailst
            for bp in range(GSZ):
                cuts = MPS_SPLITS[bp]
                for c0, c1 in zip(cuts[:-1], cuts[1:]):
                    nc.tensor.matmul(
                        mps2[:, bp, c0:c1], c_ones2[:],
                        mrow2[:, bp * DIM + c0 : bp * DIM + c1],
                        start=True, stop=True, skip_group_check=True,
                    )

        def tail_sample(pend, tailst, bp):
            g, zs, _rs_tots, _rowsum_g = pend
            mps2, _mrow2, s0s = tailst
            b = GSZ * g + bp
            z, s0_b = zs[bp], s0s[bp]
            outt = opool.tile([128, NCHUNK, DIM], F32, tag="outt")
            mv = mps2[:, bp, :].rearrange("p (a b) -> p a b", a=NCHUNK)
            for r in range(NCHUNK):
                zv = z[:, r, :].rearrange("p (a b) -> p a b", a=NCHUNK)
                ov = outt[:, r, :].rearrange("p (b f) -> p f b", f=NCHUNK)
                nc.vector.scalar_tensor_tensor(
                    ov, zv, s0_b[:, r : r + 1], mv,
                    op0=OP.subtract, op1=OP.subtract,
                )
                nc.sync.dma_start(
                    out[b].rearrange("(p j c) -> p j c", p=128, j=NCHUNK)[:, r, :],
                    outt[:, r, :],
                )

        def emit_sample_c(head, bp, z):
            xTs, bias_g, hrow2, rowsum_g = head
            xT = xTs[bp]
            for r in range(NCHUNK):
                lhsT = xT[:, r * 128 : (r + 1) * 128]
                ps = ps_g.tile([128, DIM], F32, tag="gram")
                nc.tensor.matmul(
                    ps[:, 0:512], lhsT, xT[:, 0:512],
                    start=True, stop=False, skip_group_check=True,
                )
                nc.tensor.matmul(
                    ps[:, 512:640], lhsT, xT[:, 512:640],
                    start=True, stop=False, skip_group_check=True,
                )
                nc.tensor.matmul(
                    ps[:, 0:512], c_ones2[:],
                    hrow2[:, bp * DIM : bp * DIM + 512],
                    start=False, stop=True, skip_group_check=True,
                )
                nc.tensor.matmul(
                    ps[:, 512:640], c_ones2[:],
                    hrow2[:, bp * DIM + 512 : (bp + 1) * DIM],
                    start=False, stop=True, skip_group_check=True,
                )
                nc.scalar.activation(
                    z[:, r, :], ps[:], AF.Sqrt,
                    bias=bias_g[:, NCHUNK * bp + r : NCHUNK * bp + r + 1],
                    scale=neg2alpha,
                    accum_out=rowsum_g[:, NCHUNK * bp + r : NCHUNK * bp + r + 1],
                )

        xbgs = {0: in_dma(0), 1: in_dma(1)}
        pending = None
        for g in range(NG):
            # tail-critical DVE/PE work for g-1 first, so its queue slots
            # are not stuck behind this group's head ops
            tailst = tail_prep(pending) if pending is not None else None
            head = emit_head(g, xbgs.pop(g))
            if g + 2 < NG:
                xbgs[g + 2] = in_dma(g + 2)
            if tailst is not None:
                tail_colmean(tailst)
            zs = []
            rs_tots = []
            for bp in range(GSZ):
                z = zp.tile([128, NCHUNK, DIM], F32, tag="z")
                emit_sample_c(head, bp, z)
                zs.append(z)
                if pending is not None:
                    tail_sample(pending, tailst, bp)
            # rs_tot after the tail stts so they don't block DVE's queue
            for bp in range(GSZ):
                rs_tot = psamp.tile([128, 1], F32, tag="rs")
                nc.vector.tensor_reduce(
                    rs_tot[:], head[3][:, NCHUNK * bp : NCHUNK * (bp + 1)],
                    axis=mybir.AxisListType.X, op=OP.add,
                )
                rs_tots.append(rs_tot)
            pending = (g, zs, rs_tots, head[3])

        tailst = tail_prep(pending)
        tail_colmean(tailst)
        for bp in range(GSZ):
            tail_sample(pending, tailst, bp)

    nc.compile()
    return nc


def _get_nc():
    global _cached_nc
    if _cached_nc is None:
        _cached_nc = build()
    return _cached_nc


def make_in_maps(x: np.ndarray, t: np.ndarray):
    alpha = float(np.exp(t.astype(np.float64))[0, 0])
    consts = np.zeros((128, 2), dtype=np.float32)
    consts[:, 0] = -2.0 * alpha
    consts[:, 1] = 2.0 * alpha
    ident = np.eye(128, dtype=np.float32)
    xs = x.reshape(N_CORES, B_CORE, DIM, M)
    return [
        {"x": np.ascontiguousarray(xs[c]), "consts": consts, "ident": ident}
        for c in range(N_CORES)
    ]


def kernel(x: np.ndarray, t: np.ndarray) -> np.ndarray:
    x = np.asarray(x, dtype=np.float32)
    t = np.asarray(t, dtype=np.float32)
    nc = _get_nc()
    res = run_bass_kernel_spmd(nc, make_in_maps(x, t), core_ids=list(range(N_CORES)))
    return np.concatenate([r["out"] for r in res.results], axis=0)


# revision 21
# speedup vs baseline: 1.0841x; 1.0746x over previous
"""Trainium2 Bass kernel for per-sample Brownian-distance-covariance (BDC) pooling.

Problem: x [128, 640, 100] f32, t [1,1] f32 (log temperature).
  per sample: G = x @ x^T; dcov = d_i + d_j - 2G; dcov = max(dcov, 1e-4);
  z = sqrt(exp(t)*dcov + 1e-5); out = z - rowmean - colmean + totmean.
Output: [128, 409600] f32.

Strategy (8 NeuronCores, pure data parallel, 16 samples/core), v2:
  - Row-coalesced input layout: partition p holds dims 5p..5p+4, so the
    f32->bf16 cast load is one big-descriptor DMA per group of 2 samples.
  - d = ||x_i||^2 via Pool-engine square + DVE segmented reduce (keeps the
    Activation engine free for the sqrt pass, which is its roofline).
  - Gram via TensorE with the d_j rank-2 (bf16 hi/lo) row folded into the
    same PSUM accumulation; d_i enters via the per-partition activation
    bias, which also compensates bf16 rounding exactly on the diagonal.
  - hrow/mrow row-vectors are packed with ONE flat SBUF->SBUF DMA per group
    (t5/t10 [2NQ,128] flattens to [2, NQ*128] in natural order), packed in
    the free dim so all matmul operands start at partition 0.
  - Double centering: one scalar_tensor_tensor pass per chunk, split
    between DVE and Pool, reading the colmean broadcast directly from PSUM;
    its output AP de-permutes columns so each sample's result is a single
    contiguous [128, 3200] tile, stored with ONE output DMA per sample.
"""
import numpy as np
from contextlib import ExitStack

import concourse.bass as bass
import concourse.bacc as bacc
import concourse.tile as tile
from concourse import mybir
from concourse.bass_utils import run_bass_kernel_spmd

F32 = mybir.dt.float32
BF16 = mybir.dt.bfloat16
AF = mybir.ActivationFunctionType
OP = mybir.AluOpType

N_CORES = 8
B_TOTAL = 128
B_CORE = B_TOTAL // N_CORES  # 16
DIM = 640
M = 100
NCHUNK = DIM // 128  # 5
GSZ = 2
NG = B_CORE // GSZ  # 8 groups
NQ = NCHUNK * GSZ  # 10
# bank-safe col splits for the colmean matmul, per sample slot in mps2:
# slot bp starts at byte 2560*bp inside the PSUM tile, so bp=1 must split
# at the 4096-byte bank boundary (matmul out cannot cross a PSUM bank).
MPS_SPLITS = {0: (0, 512, 640), 1: (0, 384, 512, 640)}

_cached_nc = None


def build():
    nc = bacc.Bacc("TRN2", target_bir_lowering=False)
    x = nc.dram_tensor("x", [B_CORE, DIM, M], F32, kind="ExternalInput")
    consts = nc.dram_tensor("consts", [128, 2], F32, kind="ExternalInput")
    ident_in = nc.dram_tensor("ident", [128, 128], F32, kind="ExternalInput")
    out = nc.dram_tensor("out", [B_CORE, DIM * DIM], F32, kind="ExternalOutput")

    with tile.TileContext(nc) as tc, ExitStack() as ctx:
        const_p = ctx.enter_context(tc.tile_pool(name="const", bufs=1))
        xbp = ctx.enter_context(tc.tile_pool(name="xbp", bufs=3))
        sqp = ctx.enter_context(tc.tile_pool(name="sqp", bufs=2))
        gp = ctx.enter_context(tc.tile_pool(name="gp", bufs=3))
        xtp = ctx.enter_context(tc.tile_pool(name="xtp", bufs=5))
        zp = ctx.enter_context(tc.tile_pool(name="zp", bufs=6))
        opool = ctx.enter_context(tc.tile_pool(name="op", bufs=3))
        pk = ctx.enter_context(tc.tile_pool(name="pk", bufs=3))
        psamp = ctx.enter_context(tc.tile_pool(name="psamp", bufs=6))
        ps_g = ctx.enter_context(tc.tile_pool(name="psg", bufs=2, space="PSUM"))
        ps_m = ctx.enter_context(tc.tile_pool(name="psm", bufs=1, space="PSUM"))
        ps_x = ctx.enter_context(tc.tile_pool(name="psx", bufs=1, space="PSUM"))

        # ---- constants ----
        c_consts = const_p.tile([128, 2], F32)
        nc.sync.dma_start(c_consts[:], consts[:])
        neg2alpha = c_consts[:, 0:1]
        twoalpha = c_consts[:, 1:2]

        c_ident = const_p.tile([128, 128], BF16)
        nc.gpsimd.dma_start(c_ident[:], ident_in[:])

        c_ones2 = const_p.tile([2, 128], BF16)
        nc.vector.memset(c_ones2[:], 1.0)
        c_ones128 = const_p.tile([128, 128], F32)
        nc.vector.memset(c_ones128[:], 1.0)

        def in_dma(g):
            b0 = GSZ * g
            xbg = xbp.tile([128, GSZ, NCHUNK, M], BF16, tag="xb")
            nc.gpsimd.dma_start(
                xbg[:],
                x[b0 : b0 + GSZ].rearrange("s (p r) m -> p s r m", p=128),
            )
            return xbg

        def emit_head(g, xbg):
            # squares + segmented reduce -> d  [128, NQ]
            sqg = sqp.tile([128, GSZ, NCHUNK, M], F32, tag="sq")
            nc.gpsimd.tensor_mul(sqg[:], xbg[:], xbg[:])
            # per-sample transposes -> xT [100, 640]
            xTs = []
            for bp in range(GSZ):
                xps = ps_x.tile([M, DIM], BF16, tag="xps")
                for r in range(NCHUNK):
                    nc.tensor.transpose(
                        xps[:, r * 128 : (r + 1) * 128], xbg[:, bp, r, :], c_ident[:]
                    )
                xT = xtp.tile([M, DIM], BF16, tag="xT")
                nc.vector.tensor_copy(xT[:], xps[:])
                xTs.append(xT)
            d_g = gp.tile([128, NQ], F32, tag="d")
            nc.vector.tensor_reduce(
                d_g[:], sqg[:], axis=mybir.AxisListType.X, op=OP.add
            )
            # hi/lo split of -0.5*d, bias
            hstack = gp.tile([128, 2 * NQ], BF16, tag="hstack")
            nc.vector.tensor_scalar(
                out=hstack[:, 0:NQ], in0=d_g[:], scalar1=-0.5, scalar2=None,
                op0=OP.mult,
            )
            hres = gp.tile([128, NQ], F32, tag="hres")
            nc.vector.tensor_scalar(
                out=hres[:], in0=d_g[:], scalar1=-0.5, scalar2=None, op0=OP.mult
            )
            nc.vector.tensor_sub(hres[:], hres[:], hstack[:, 0:NQ])
            nc.vector.tensor_copy(hstack[:, NQ : 2 * NQ], hres[:])
            # transpose hi/lo stack and pack the per-sample [2, 640] rows
            xps2 = ps_x.tile([M, DIM], BF16, tag="xps")
            nc.tensor.transpose(xps2[0 : 2 * NQ, 0:128], hstack[:], c_ident[:])
            t5 = gp.tile([2 * NQ, 128], BF16, tag="t5")
            nc.vector.tensor_copy(t5[:], xps2[0 : 2 * NQ, 0:128])
            hrow2 = pk.tile([2, GSZ * DIM], BF16, tag="hrow")
            nc.sync.dma_start(hrow2[:], t5[:])
            # bias branch off the critical t5 chain (Pool adds)
            tmpb = gp.tile([128, NQ], F32, tag="tmpb")
            nc.gpsimd.tensor_add(tmpb[:], d_g[:], hstack[:, 0:NQ])
            nc.gpsimd.tensor_add(tmpb[:], tmpb[:], hstack[:, NQ : 2 * NQ])
            bias_g = gp.tile([128, NQ], F32, tag="bias")
            nc.vector.tensor_scalar(
                out=bias_g[:], in0=tmpb[:], scalar1=twoalpha, scalar2=1e-5,
                op0=OP.mult, op1=OP.add,
            )
            rowsum_g = gp.tile([128, NQ], F32, tag="rowsum")
            return xTs, bias_g, hrow2, rowsum_g

        def tail_prep(pend):
            _g, _zs, rs_tots, rowsum_g = pend
            rm_g = gp.tile([128, NQ], F32, tag="rm")
            nc.vector.tensor_scalar(
                out=rm_g[:], in0=rowsum_g[:], scalar1=1.0 / DIM, scalar2=None,
                op0=OP.mult,
            )
            rmstack = gp.tile([128, 2 * NQ], BF16, tag="rmstack")
            nc.vector.tensor_copy(rmstack[:, 0:NQ], rm_g[:])
            rml = gp.tile([128, NQ], F32, tag="rml")
            nc.vector.tensor_sub(rml[:], rm_g[:], rmstack[:, 0:NQ])
            nc.vector.tensor_copy(rmstack[:, NQ : 2 * NQ], rml[:])
            xps3 = ps_x.tile([M, DIM], BF16, tag="xps")
            nc.tensor.transpose(xps3[0 : 2 * NQ, 0:128], rmstack[:], c_ident[:])
            t10 = gp.tile([2 * NQ, 128], BF16, tag="t10")
            nc.vector.tensor_copy(t10[:], xps3[0 : 2 * NQ, 0:128])
            mrow2 = pk.tile([2, GSZ * DIM], BF16, tag="mrow")
            nc.sync.dma_start(mrow2[:], t10[:])
            mps2 = ps_m.tile([128, GSZ, DIM], F32, tag="mps")
            s0s = []
            for bp in range(GSZ):
                nc.tensor.matmul(
                    mps2[:, bp, 0:1], c_ones128[:], rs_tots[bp][:],
                    start=True, stop=True, skip_group_check=True,
                )
                tm_b = psamp.tile([128, 1], F32, tag="tm")
                nc.scalar.mul(tm_b[:], mps2[:, bp, 0:1], 1.0 / (DIM * DIM))
                s0_b = psamp.tile([128, NCHUNK], F32, tag="s0")
                nc.vector.tensor_scalar(
                    out=s0_b[:], in0=rm_g[:, NCHUNK * bp : NCHUNK * (bp + 1)],
                    scalar1=tm_b[:], scalar2=None, op0=OP.subtract,
                )
                s0s.append(s0_b)
            return mps2, mrow2, s0s

        def tail_colmean(tailst):
            mps2, mrow2, _s0s = tailst
            for bp in range(GSZ):
                cuts = MPS_SPLITS[bp]
                for c0, c1 in zip(cuts[:-1], cuts[1:]):
                    nc.tensor.matmul(
                        mps2[:, bp, c0:c1], c_ones2[:],
                        mrow2[:, bp * DIM + c0 : bp * DIM + c1],
                        start=True, stop=True, skip_group_check=True,
                    )

        def tail_sample(pend, tailst, bp):
            g, zs, _rs_tots, _rowsum_g = pend
            mps2, _mrow2, s0s = tailst
            b = GSZ * g + bp
            z, s0_b = zs[bp], s0s[bp]
            outt = opool.tile([128, NCHUNK, DIM], F32, tag="outt")
            mv = mps2[:, bp, :].rearrange("p (a b) -> p a b", a=NCHUNK)
            for r in range(NCHUNK):
                zv = z[:, r, :].rearrange("p (a b) -> p a b", a=NCHUNK)
                ov = outt[:, r, :].rearrange("p (b f) -> p f b", f=NCHUNK)
                nc.vector.scalar_tensor_tensor(
                    ov, zv, s0_b[:, r : r + 1], mv,
                    op0=OP.subtract, op1=OP.subtract,
                )
                nc.sync.dma_start(
                    out[b].rearrange("(p j c) -> p j c", p=128, j=NCHUNK)[:, r, :],
                    outt[:, r, :],
                )

        def emit_sample_c(head, bp, z):
            xTs, bias_g, hrow2, rowsum_g = head
            xT = xTs[bp]
            for r in range(NCHUNK):
                lhsT = xT[:, r * 128 : (r + 1) * 128]
                ps = ps_g.tile([128, DIM], F32, tag="gram")
                nc.tensor.matmul(
                    ps[:, 0:512], lhsT, xT[:, 0:512],
                    start=True, stop=False, skip_group_check=True,
                )
                nc.tensor.matmul(
                    ps[:, 512:640], lhsT, xT[:, 512:640],
                    start=True, stop=False, skip_group_check=True,
                )
                nc.tensor.matmul(
                    ps[:, 0:512], c_ones2[:],
                    hrow2[:, bp * DIM : bp * DIM + 512],
                    start=False, stop=True, skip_group_check=True,
                )
                nc.tensor.matmul(
                    ps[:, 512:640], c_ones2[:],
                    hrow2[:, bp * DIM + 512 : (bp + 1) * DIM],
                    start=False, stop=True, skip_group_check=True,
                )
                nc.scalar.activation(
                    z[:, r, :], ps[:], AF.Sqrt,
                    bias=bias_g[:, NCHUNK * bp + r : NCHUNK * bp + r + 1],
                    scale=neg2alpha,
                    accum_out=rowsum_g[:, NCHUNK * bp + r : NCHUNK * bp + r + 1],
                )

        # software pipeline, one iteration = one group's compute (phase C +
        # previous group's tail), with the NEXT group's head prepared a full
        # iteration ahead so gram/sqrt inputs are never on the critical path.
        xbgs = {0: in_dma(0), 1: in_dma(1), 2: in_dma(2)}
        heads = {0: emit_head(0, xbgs.pop(0))}
        pending = None
        for g in range(NG):
            # tail-critical chain for g-1 first: its DVE/PE inputs are ready
            tailst = tail_prep(pending) if pending is not None else None
            # next group's head (uses xbg prefetched 2-3 iterations ago)
            if g + 1 < NG:
                heads[g + 1] = emit_head(g + 1, xbgs.pop(g + 1))
            if g + 3 < NG:
                xbgs[g + 3] = in_dma(g + 3)
            if tailst is not None:
                tail_colmean(tailst)
            head = heads.pop(g)
            zs = []
            rs_tots = []
            for bp in range(GSZ):
                z = zp.tile([128, NCHUNK, DIM], F32, tag="z")
                emit_sample_c(head, bp, z)
                zs.append(z)
                if pending is not None:
                    tail_sample(pending, tailst, bp)
                rs_tot = psamp.tile([128, 1], F32, tag="rs")
                nc.vector.tensor_reduce(
                    rs_tot[:], head[3][:, NCHUNK * bp : NCHUNK * (bp + 1)],
                    axis=mybir.AxisListType.X, op=OP.add,
                )
                rs_tots.append(rs_tot)
            pending = (g, zs, rs_tots, head[3])

        tailst = tail_prep(pending)
        tail_colmean(tailst)
        for bp in range(GSZ):
            tail_sample(pending, tailst, bp)

    nc.compile()
    return nc


def _get_nc():
    global _cached_nc
    if _cached_nc is None:
        _cached_nc = build()
    return _cached_nc


def make_in_maps(x: np.ndarray, t: np.ndarray):
    alpha = float(np.exp(t.astype(np.float64))[0, 0])
    consts = np.zeros((128, 2), dtype=np.float32)
    consts[:, 0] = -2.0 * alpha
    consts[:, 1] = 2.0 * alpha
    ident = np.eye(128, dtype=np.float32)
    xs = x.reshape(N_CORES, B_CORE, DIM, M)
    return [
        {"x": np.ascontiguousarray(xs[c]), "consts": consts, "ident": ident}
        for c in range(N_CORES)
    ]


def kernel(x: np.ndarray, t: np.ndarray) -> np.ndarray:
    x = np.asarray(x, dtype=np.float32)
    t = np.asarray(t, dtype=np.float32)
    nc = _get_nc()
    res = run_bass_kernel_spmd(nc, make_in_maps(x, t), core_ids=list(range(N_CORES)))
    return np.concatenate([r["out"] for r in res.results], axis=0)


# revision 24
# speedup vs baseline: 1.0944x; 1.0095x over previous
"""Trainium2 Bass kernel for per-sample Brownian-distance-covariance (BDC) pooling.

Problem: x [128, 640, 100] f32, t [1,1] f32 (log temperature).
  per sample: G = x @ x^T; dcov = d_i + d_j - 2G; dcov = max(dcov, 1e-4);
  z = sqrt(exp(t)*dcov + 1e-5); out = z - rowmean - colmean + totmean.
Output: [128, 409600] f32.

Strategy (8 NeuronCores, pure data parallel, 16 samples/core), v3:
  - Row-coalesced input layout: partition p holds dims 5p..5p+4, so the
    f32->bf16 cast load is one big-descriptor DMA per group of 2 samples.
  - d = ||x_i||^2 via Pool-engine square + DVE segmented reduce (keeps the
    Activation engine free for the sqrt pass, which is near its roofline).
  - Gram via TensorE with the d_j rank-2 (bf16 hi/lo) row folded into the
    same PSUM accumulation; d_i enters via the per-partition activation
    bias, which also compensates bf16 rounding exactly on the diagonal.
  - hrow/mrow row-vectors are packed with ONE flat SBUF->SBUF DMA each
    (t5/t10 [k,128] flattens to [2, k/2*128] in natural order), packed in
    the free dim so all matmul operands start at partition 0.
  - Sample-level software pipeline: tail of sample n-1 (double centering +
    store) overlaps gram+sqrt of sample n; group-level head work (d, hi/lo
    split, hrow pack) runs a full group ahead.
  - Double centering: one scalar_tensor_tensor pass per chunk on DVE,
    reading the colmean broadcast directly from PSUM; its output AP
    de-permutes columns, and each [128,640] chunk is stored with its own
    DMA so the serialized DMA engine interleaves small critical transfers.
"""
import numpy as np
from contextlib import ExitStack

import concourse.bass as bass
import concourse.bacc as bacc
import concourse.tile as tile
from concourse import mybir
from concourse.bass_utils import run_bass_kernel_spmd

F32 = mybir.dt.float32
BF16 = mybir.dt.bfloat16
AF = mybir.ActivationFunctionType
OP = mybir.AluOpType

N_CORES = 8
B_TOTAL = 128
B_CORE = B_TOTAL // N_CORES  # 16
DIM = 640
M = 100
NCHUNK = DIM // 128  # 5
GSZ = 2
NG = B_CORE // GSZ  # 8 groups
NQ = NCHUNK * GSZ  # 10
# bank-safe col splits for the colmean matmul, per sample slot in mps2:
# slot bp starts at byte 2560*bp inside the PSUM tile, so bp=1 must split
# at the 4096-byte bank boundary (matmul out cannot cross a PSUM bank).
MPS_SPLITS = {0: (0, 512, 640), 1: (0, 384, 512, 640)}

_cached_nc = None


def build():
    nc = bacc.Bacc("TRN2", target_bir_lowering=False)
    x = nc.dram_tensor("x", [B_CORE, DIM, M], F32, kind="ExternalInput")
    consts = nc.dram_tensor("consts", [128, 2], F32, kind="ExternalInput")
    ident_in = nc.dram_tensor("ident", [128, 128], F32, kind="ExternalInput")
    out = nc.dram_tensor("out", [B_CORE, DIM * DIM], F32, kind="ExternalOutput")

    with tile.TileContext(nc) as tc, ExitStack() as ctx:
        const_p = ctx.enter_context(tc.tile_pool(name="const", bufs=1))
        xbp = ctx.enter_context(tc.tile_pool(name="xbp", bufs=4))
        sqp = ctx.enter_context(tc.tile_pool(name="sqp", bufs=2))
        gp = ctx.enter_context(tc.tile_pool(name="gp", bufs=3))
        xtp = ctx.enter_context(tc.tile_pool(name="xtp", bufs=5))
        zp = ctx.enter_context(tc.tile_pool(name="zp", bufs=5))
        opool = ctx.enter_context(tc.tile_pool(name="op", bufs=3))
        pk = ctx.enter_context(tc.tile_pool(name="pk", bufs=3))
        psamp = ctx.enter_context(tc.tile_pool(name="psamp", bufs=4))
        ps_g = ctx.enter_context(tc.tile_pool(name="psg", bufs=2, space="PSUM"))
        ps_m = ctx.enter_context(tc.tile_pool(name="psm", bufs=1, space="PSUM"))
        ps_x = ctx.enter_context(tc.tile_pool(name="psx", bufs=1, space="PSUM"))

        # ---- constants ----
        c_consts = const_p.tile([128, 2], F32)
        nc.sync.dma_start(c_consts[:], consts[:])
        neg2alpha = c_consts[:, 0:1]
        twoalpha = c_consts[:, 1:2]

        c_ident = const_p.tile([128, 128], BF16)
        nc.gpsimd.dma_start(c_ident[:], ident_in[:])

        c_ones2 = const_p.tile([2, 128], BF16)
        nc.vector.memset(c_ones2[:], 1.0)
        c_ones128 = const_p.tile([128, 128], F32)
        nc.vector.memset(c_ones128[:], 1.0)

        def in_dma(g):
            b0 = GSZ * g
            xbg = xbp.tile([128, GSZ, NCHUNK, M], BF16, tag="xb")
            nc.gpsimd.dma_start(
                xbg[:],
                x[b0 : b0 + GSZ].rearrange("s (p r) m -> p s r m", p=128),
            )
            return xbg

        def emit_head(g, xbg):
            # squares + segmented reduce -> d  [128, NQ]
            sqg = sqp.tile([128, GSZ, NCHUNK, M], F32, tag="sq")
            nc.gpsimd.tensor_mul(sqg[:], xbg[:], xbg[:])
            # per-sample transposes -> xT [100, 640]
            xTs = []
            for bp in range(GSZ):
                xps = ps_x.tile([M, DIM], BF16, tag="xps")
                for r in range(NCHUNK):
                    nc.tensor.transpose(
                        xps[:, r * 128 : (r + 1) * 128], xbg[:, bp, r, :], c_ident[:]
                    )
                xT = xtp.tile([M, DIM], BF16, tag="xT")
                nc.vector.tensor_copy(xT[:], xps[:])
                xTs.append(xT)
            d_g = gp.tile([128, NQ], F32, tag="d")
            nc.vector.tensor_reduce(
                d_g[:], sqg[:], axis=mybir.AxisListType.X, op=OP.add
            )
            # hi/lo split of -0.5*d
            hstack = gp.tile([128, 2 * NQ], BF16, tag="hstack")
            nc.vector.tensor_scalar(
                out=hstack[:, 0:NQ], in0=d_g[:], scalar1=-0.5, scalar2=None,
                op0=OP.mult,
            )
            hres = gp.tile([128, NQ], F32, tag="hres")
            nc.vector.tensor_scalar(
                out=hres[:], in0=d_g[:], scalar1=-0.5, scalar2=None, op0=OP.mult
            )
            nc.vector.tensor_sub(hres[:], hres[:], hstack[:, 0:NQ])
            nc.vector.tensor_copy(hstack[:, NQ : 2 * NQ], hres[:])
            # transpose hi/lo stack and pack the per-sample [2, 640] rows
            xps2 = ps_x.tile([M, DIM], BF16, tag="xps")
            nc.tensor.transpose(xps2[0 : 2 * NQ, 0:128], hstack[:], c_ident[:])
            t5 = gp.tile([2 * NQ, 128], BF16, tag="t5")
            nc.vector.tensor_copy(t5[:], xps2[0 : 2 * NQ, 0:128])
            hrow2 = pk.tile([2, GSZ * DIM], BF16, tag="hrow")
            nc.sync.dma_start(hrow2[:], t5[:])
            # bias branch off the critical t5 chain (Pool adds)
            tmpb = gp.tile([128, NQ], F32, tag="tmpb")
            nc.gpsimd.tensor_add(tmpb[:], d_g[:], hstack[:, 0:NQ])
            nc.gpsimd.tensor_add(tmpb[:], tmpb[:], hstack[:, NQ : 2 * NQ])
            bias_g = gp.tile([128, NQ], F32, tag="bias")
            nc.vector.tensor_scalar(
                out=bias_g[:], in0=tmpb[:], scalar1=twoalpha, scalar2=1e-5,
                op0=OP.mult, op1=OP.add,
            )
            rowsum_g = gp.tile([128, NQ], F32, tag="rowsum")
            return xTs, bias_g, hrow2, rowsum_g

        mps2_hold = [None]

        def tail_prep(st):
            """Per-sample tail prep: rowmean, hi/lo split, mrow pack,
            total/colmean matmuls (colmean deferred to tail_colmean)."""
            rowsum_g, rs_tot, bp = st["rowsum"], st["rs"], st["bp"]
            rm_s = psamp.tile([128, NCHUNK], F32, tag="rm")
            nc.vector.tensor_scalar(
                out=rm_s[:], in0=rowsum_g[:, NCHUNK * bp : NCHUNK * (bp + 1)],
                scalar1=1.0 / DIM, scalar2=None, op0=OP.mult,
            )
            rmstack = psamp.tile([128, 2 * NCHUNK], BF16, tag="rmstack")
            nc.vector.tensor_copy(rmstack[:, 0:NCHUNK], rm_s[:])
            rml = psamp.tile([128, NCHUNK], F32, tag="rml")
            nc.vector.tensor_sub(rml[:], rm_s[:], rmstack[:, 0:NCHUNK])
            nc.vector.tensor_copy(rmstack[:, NCHUNK : 2 * NCHUNK], rml[:])
            xps3 = ps_x.tile([M, DIM], BF16, tag="xps")
            nc.tensor.transpose(
                xps3[0 : 2 * NCHUNK, 0:128], rmstack[:], c_ident[:]
            )
            t10 = psamp.tile([2 * NCHUNK, 128], BF16, tag="t10")
            nc.vector.tensor_copy(t10[:], xps3[0 : 2 * NCHUNK, 0:128])
            mrow1 = pk.tile([2, DIM], BF16, tag="mrow")
            nc.sync.dma_start(mrow1[:], t10[:])
            if bp == 0:
                mps2 = ps_m.tile([128, GSZ, DIM], F32, tag="mps")
                mps2_hold[0] = mps2
            mps2 = mps2_hold[0]
            nc.tensor.matmul(
                mps2[:, bp, 0:1], c_ones128[:], rs_tot[:],
                start=True, stop=True, skip_group_check=True,
            )
            tm_b = psamp.tile([128, 1], F32, tag="tm")
            nc.scalar.mul(tm_b[:], mps2[:, bp, 0:1], 1.0 / (DIM * DIM))
            s0_b = psamp.tile([128, NCHUNK], F32, tag="s0")
            nc.vector.tensor_scalar(
                out=s0_b[:], in0=rm_s[:], scalar1=tm_b[:], scalar2=None,
                op0=OP.subtract,
            )
            st["mps2"], st["mrow"], st["s0"] = mps2, mrow1, s0_b

        def tail_colmean(st):
            mps2, mrow1, bp = st["mps2"], st["mrow"], st["bp"]
            cuts = MPS_SPLITS[bp]
            for c0, c1 in zip(cuts[:-1], cuts[1:]):
                nc.tensor.matmul(
                    mps2[:, bp, c0:c1], c_ones2[:], mrow1[:, c0:c1],
                    start=True, stop=True, skip_group_check=True,
                )

        def tail_stt(st):
            b, bp, z = st["n"], st["bp"], st["z"]
            mps2, s0_b = st["mps2"], st["s0"]
            outt = opool.tile([128, NCHUNK, DIM], F32, tag="outt")
            mv = mps2[:, bp, :].rearrange("p (a b) -> p a b", a=NCHUNK)
            for r in range(NCHUNK):
                zv = z[:, r, :].rearrange("p (a b) -> p a b", a=NCHUNK)
                ov = outt[:, r, :].rearrange("p (b f) -> p f b", f=NCHUNK)
                nc.vector.scalar_tensor_tensor(
                    ov, zv, s0_b[:, r : r + 1], mv,
                    op0=OP.subtract, op1=OP.subtract,
                )
                nc.sync.dma_start(
                    out[b].rearrange("(p j c) -> p j c", p=128, j=NCHUNK)[:, r, :],
                    outt[:, r, :],
                )

        def emit_sample_c(head, bp, z, mid_cb=None):
            xTs, bias_g, hrow2, rowsum_g = head
            xT = xTs[bp]
            for r in range(NCHUNK):
                lhsT = xT[:, r * 128 : (r + 1) * 128]
                ps = ps_g.tile([128, DIM], F32, tag="gram")
                nc.tensor.matmul(
                    ps[:, 0:512], lhsT, xT[:, 0:512],
                    start=True, stop=False, skip_group_check=True,
                )
                nc.tensor.matmul(
                    ps[:, 512:640], lhsT, xT[:, 512:640],
                    start=True, stop=False, skip_group_check=True,
                )
                nc.tensor.matmul(
                    ps[:, 0:512], c_ones2[:],
                    hrow2[:, bp * DIM : bp * DIM + 512],
                    start=False, stop=True, skip_group_check=True,
                )
                nc.tensor.matmul(
                    ps[:, 512:640], c_ones2[:],
                    hrow2[:, bp * DIM + 512 : (bp + 1) * DIM],
                    start=False, stop=True, skip_group_check=True,
                )
                nc.scalar.activation(
                    z[:, r, :], ps[:], AF.Sqrt,
                    bias=bias_g[:, NCHUNK * bp + r : NCHUNK * bp + r + 1],
                    scale=neg2alpha,
                    accum_out=rowsum_g[:, NCHUNK * bp + r : NCHUNK * bp + r + 1],
                )
                if r == 1 and mid_cb is not None:
                    mid_cb()

        # ---- sample-level software pipeline ----
        xbgs = {0: in_dma(0), 1: in_dma(1), 2: in_dma(2)}
        heads = {0: emit_head(0, xbgs.pop(0))}
        prev = None
        for n in range(B_CORE):
            g, bp = divmod(n, 2)
            if bp == 0 and g + 1 < NG:
                heads[g + 1] = emit_head(g + 1, xbgs.pop(g + 1))
                if g + 3 < NG:
                    xbgs[g + 3] = in_dma(g + 3)
            head = heads[g]
            if prev is not None:
                tail_prep(prev)
            z = zp.tile([128, NCHUNK, DIM], F32, tag="z")
            mid = (lambda p=prev: tail_colmean(p)) if prev is not None else None
            emit_sample_c(head, bp, z, mid)
            if prev is not None:
                tail_stt(prev)
            rs_tot = psamp.tile([128, 1], F32, tag="rs")
            nc.vector.tensor_reduce(
                rs_tot[:], head[3][:, NCHUNK * bp : NCHUNK * (bp + 1)],
                axis=mybir.AxisListType.X, op=OP.add,
            )
            prev = {"n": n, "bp": bp, "z": z, "rowsum": head[3], "rs": rs_tot}
            if bp == 1:
                heads.pop(g)

        tail_prep(prev)
        tail_colmean(prev)
        tail_stt(prev)

    nc.compile()
    return nc


def _get_nc():
    global _cached_nc
    if _cached_nc is None:
        _cached_nc = build()
    return _cached_nc


def make_in_maps(x: np.ndarray, t: np.ndarray):
    alpha = float(np.exp(t.astype(np.float64))[0, 0])
    consts = np.zeros((128, 2), dtype=np.float32)
    consts[:, 0] = -2.0 * alpha
    consts[:, 1] = 2.0 * alpha
    ident = np.eye(128, dtype=np.float32)
    xs = x.reshape(N_CORES, B_CORE, DIM, M)
    return [
        {"x": np.ascontiguousarray(xs[c]), "consts": consts, "ident": ident}
        for c in range(N_CORES)
    ]


def kernel(x: np.ndarray, t: np.ndarray) -> np.ndarray:
    x = np.asarray(x, dtype=np.float32)
    t = np.asarray(t, dtype=np.float32)
    nc = _get_nc()
    res = run_bass_kernel_spmd(nc, make_in_maps(x, t), core_ids=list(range(N_CORES)))
    return np.concatenate([r["out"] for r in res.results], axis=0)


# revision 26
# speedup vs baseline: 1.1639x; 1.0635x over previous
"""Trainium2 Bass kernel for per-sample Brownian-distance-covariance (BDC) pooling.

Problem: x [128, 640, 100] f32, t [1,1] f32 (log temperature).
  per sample: G = x @ x^T; dcov = d_i + d_j - 2G; dcov = max(dcov, 1e-4);
  z = sqrt(exp(t)*dcov + 1e-5); out = z - rowmean - colmean + totmean.
Output: [128, 409600] f32.

Strategy (8 NeuronCores, pure data parallel, 16 samples/core), v5:
  - Row-coalesced input layout: partition p holds dims 5p..5p+4, so the
    f32->bf16 cast load is one big-descriptor DMA per group of 2 samples.
  - d = ||x_i||^2 via Pool-engine square + DVE segmented reduce (keeps the
    Activation engine free for the sqrt pass, which is near its roofline).
  - Gram via TensorE with the d_j rank-2 (bf16 hi/lo) row folded into the
    same PSUM accumulation; d_i enters via the per-partition activation
    bias, which also compensates bf16 rounding exactly on the diagonal.
  - hrow/mrow row-vectors packed with ONE flat SBUF->SBUF DMA per sample
    ([k,128] transpose output flattens to [2, 640] in natural order), in
    the free dim so all matmul operands start at partition 0.
  - Fully per-sample software pipeline: head (d/hi-lo/hrow) runs 2 samples
    ahead; tail of sample n-1 (double centering + store) overlaps
    gram+sqrt of sample n; per-chunk output DMAs keep the serialized DMA
    engine interleaving small critical transfers.
  - Double centering: one scalar_tensor_tensor per chunk on DVE, reading
    the colmean broadcast from PSUM; output AP de-permutes columns.
"""
import numpy as np
from contextlib import ExitStack

import concourse.bass as bass
import concourse.bacc as bacc
import concourse.tile as tile
from concourse import mybir
from concourse.bass_utils import run_bass_kernel_spmd

F32 = mybir.dt.float32
BF16 = mybir.dt.bfloat16
AF = mybir.ActivationFunctionType
OP = mybir.AluOpType

N_CORES = 8
B_TOTAL = 128
B_CORE = B_TOTAL // N_CORES  # 16
DIM = 640
M = 100
NCHUNK = DIM // 128  # 5
GSZ = 2
NG = B_CORE // GSZ  # 8 groups

_cached_nc = None


def build():
    nc = bacc.Bacc("TRN2", target_bir_lowering=False)
    x = nc.dram_tensor("x", [B_CORE, DIM, M], F32, kind="ExternalInput")
    consts = nc.dram_tensor("consts", [128, 2], F32, kind="ExternalInput")
    ident_in = nc.dram_tensor("ident", [128, 128], F32, kind="ExternalInput")
    out = nc.dram_tensor("out", [B_CORE, DIM * DIM], F32, kind="ExternalOutput")

    with tile.TileContext(nc) as tc, ExitStack() as ctx:
        const_p = ctx.enter_context(tc.tile_pool(name="const", bufs=1))
        xbp = ctx.enter_context(tc.tile_pool(name="xbp", bufs=4))
        sqp = ctx.enter_context(tc.tile_pool(name="sqp", bufs=3))
        hp = ctx.enter_context(tc.tile_pool(name="hp", bufs=6))
        xtp = ctx.enter_context(tc.tile_pool(name="xtp", bufs=5))
        zp = ctx.enter_context(tc.tile_pool(name="zp", bufs=5))
        opool = ctx.enter_context(tc.tile_pool(name="op", bufs=3))
        pk = ctx.enter_context(tc.tile_pool(name="pk", bufs=4))
        psamp = ctx.enter_context(tc.tile_pool(name="psamp", bufs=4))
        ps_g = ctx.enter_context(tc.tile_pool(name="psg", bufs=2, space="PSUM"))
        ps_m = ctx.enter_context(tc.tile_pool(name="psm", bufs=1, space="PSUM"))
        ps_x = ctx.enter_context(tc.tile_pool(name="psx", bufs=2, space="PSUM"))

        # ---- constants ----
        c_consts = const_p.tile([128, 2], F32)
        nc.sync.dma_start(c_consts[:], consts[:])
        neg2alpha = c_consts[:, 0:1]
        twoalpha = c_consts[:, 1:2]

        c_ident = const_p.tile([128, 128], BF16)
        nc.gpsimd.dma_start(c_ident[:], ident_in[:])

        c_ones2 = const_p.tile([2, 128], BF16)
        nc.vector.memset(c_ones2[:], 1.0)
        c_ones128 = const_p.tile([128, 128], F32)
        nc.vector.memset(c_ones128[:], 1.0)

        def in_dma(g):
            b0 = GSZ * g
            xbg = xbp.tile([128, GSZ, NCHUNK, M], BF16, tag="xb")
            nc.gpsimd.dma_start(
                xbg[:],
                x[b0 : b0 + GSZ].rearrange("s (p r) m -> p s r m", p=128),
            )
            return xbg

        def emit_head(n, xbg):
            """Per-sample head: squares->d, hi/lo split, hrow pack, xT."""
            bp = n % GSZ
            sqs = sqp.tile([128, NCHUNK, M], F32, tag="sq")
            nc.gpsimd.tensor_mul(sqs[:], xbg[:, bp], xbg[:, bp])
            # transposes -> xT [100, 640]
            xps = ps_x.tile([M, DIM], BF16, tag="xps")
            for r in range(NCHUNK):
                nc.tensor.transpose(
                    xps[:, r * 128 : (r + 1) * 128], xbg[:, bp, r, :], c_ident[:]
                )
            xT = xtp.tile([M, DIM], BF16, tag="xT")
            if n % 2 == 0:
                nc.scalar.copy(xT[:], xps[:])
            else:
                nc.vector.tensor_copy(xT[:], xps[:])
            d_s = hp.tile([128, NCHUNK], F32, tag="d")
            nc.vector.tensor_reduce(
                d_s[:], sqs[:], axis=mybir.AxisListType.X, op=OP.add
            )
            # hi/lo split of -0.5*d
            hstack = hp.tile([128, 2 * NCHUNK], BF16, tag="hstack")
            nc.vector.tensor_scalar(
                out=hstack[:, 0:NCHUNK], in0=d_s[:], scalar1=-0.5, scalar2=None,
                op0=OP.mult,
            )
            hres = hp.tile([128, NCHUNK], F32, tag="hres")
            nc.vector.tensor_scalar(
                out=hres[:], in0=d_s[:], scalar1=-0.5, scalar2=None, op0=OP.mult
            )
            nc.vector.tensor_sub(hres[:], hres[:], hstack[:, 0:NCHUNK])
            nc.vector.tensor_copy(hstack[:, NCHUNK : 2 * NCHUNK], hres[:])
            # transpose hi/lo stack, pack [2, 640] row
            xps2 = ps_x.tile([M, DIM], BF16, tag="xps")
            nc.tensor.transpose(
                xps2[0 : 2 * NCHUNK, 0:128], hstack[:], c_ident[:]
            )
            t5 = hp.tile([2 * NCHUNK, 128], BF16, tag="t5")
            nc.vector.tensor_copy(t5[:], xps2[0 : 2 * NCHUNK, 0:128])
            hrow1 = pk.tile([2, DIM], BF16, tag="hrow")
            nc.sync.dma_start(hrow1[:], t5[:])
            # bias branch off the critical chain (Pool adds)
            tmpb = hp.tile([128, NCHUNK], F32, tag="tmpb")
            nc.gpsimd.tensor_add(tmpb[:], d_s[:], hstack[:, 0:NCHUNK])
            nc.gpsimd.tensor_add(tmpb[:], tmpb[:], hstack[:, NCHUNK : 2 * NCHUNK])
            bias_s = hp.tile([128, NCHUNK], F32, tag="bias")
            nc.vector.tensor_scalar(
                out=bias_s[:], in0=tmpb[:], scalar1=twoalpha, scalar2=1e-5,
                op0=OP.mult, op1=OP.add,
            )
            rowsum_s = hp.tile([128, NCHUNK], F32, tag="rowsum")
            return xT, bias_s, hrow1, rowsum_s

        def tail_prep(st):
            """Per-sample tail prep: rowmean, hi/lo, mrow pack, total mm."""
            rowsum_s, rs_tot = st["rowsum"], st["rs"]
            rm_s = psamp.tile([128, NCHUNK], F32, tag="rm")
            nc.vector.tensor_scalar(
                out=rm_s[:], in0=rowsum_s[:], scalar1=1.0 / DIM, scalar2=None,
                op0=OP.mult,
            )
            rmstack = psamp.tile([128, 2 * NCHUNK], BF16, tag="rmstack")
            nc.vector.tensor_copy(rmstack[:, 0:NCHUNK], rm_s[:])
            rml = psamp.tile([128, NCHUNK], F32, tag="rml")
            nc.vector.tensor_sub(rml[:], rm_s[:], rmstack[:, 0:NCHUNK])
            nc.vector.tensor_copy(rmstack[:, NCHUNK : 2 * NCHUNK], rml[:])
            xps3 = ps_x.tile([M, DIM], BF16, tag="xps")
            nc.tensor.transpose(
                xps3[0 : 2 * NCHUNK, 0:128], rmstack[:], c_ident[:]
            )
            t10 = psamp.tile([2 * NCHUNK, 128], BF16, tag="t10")
            nc.vector.tensor_copy(t10[:], xps3[0 : 2 * NCHUNK, 0:128])
            mrow1 = pk.tile([2, DIM], BF16, tag="mrow")
            nc.sync.dma_start(mrow1[:], t10[:])
            mps = ps_m.tile([128, DIM], F32, tag="mps")
            nc.tensor.matmul(
                mps[:, 0:1], c_ones128[:], rs_tot[:],
                start=True, stop=True, skip_group_check=True,
            )
            tm_b = psamp.tile([128, 1], F32, tag="tm")
            nc.scalar.mul(tm_b[:], mps[:, 0:1], 1.0 / (DIM * DIM))
            s0_b = psamp.tile([128, NCHUNK], F32, tag="s0")
            nc.vector.tensor_scalar(
                out=s0_b[:], in0=rm_s[:], scalar1=tm_b[:], scalar2=None,
                op0=OP.subtract,
            )
            st["mps"], st["mrow"], st["s0"] = mps, mrow1, s0_b

        def tail_colmean(st):
            mps, mrow1 = st["mps"], st["mrow"]
            nc.tensor.matmul(
                mps[:, 0:512], c_ones2[:], mrow1[:, 0:512],
                start=True, stop=True, skip_group_check=True,
            )
            nc.tensor.matmul(
                mps[:, 512:640], c_ones2[:], mrow1[:, 512:640],
                start=True, stop=True, skip_group_check=True,
            )

        def tail_stt(st):
            b, z = st["n"], st["z"]
            mps, s0_b = st["mps"], st["s0"]
            outt = opool.tile([128, NCHUNK, DIM], F32, tag="outt")
            mv = mps[:].rearrange("p (a b) -> p a b", a=NCHUNK)
            for r in range(NCHUNK):
                zv = z[:, r, :].rearrange("p (a b) -> p a b", a=NCHUNK)
                ov = outt[:, r, :].rearrange("p (b f) -> p f b", f=NCHUNK)
                nc.vector.scalar_tensor_tensor(
                    ov, zv, s0_b[:, r : r + 1], mv,
                    op0=OP.subtract, op1=OP.subtract,
                )
                nc.sync.dma_start(
                    out[b].rearrange("(p j c) -> p j c", p=128, j=NCHUNK)[:, r, :],
                    outt[:, r, :],
                )

        def emit_sample_c(head, z, mid_cb=None):
            xT, bias_s, hrow1, rowsum_s = head
            for r in range(NCHUNK):
                lhsT = xT[:, r * 128 : (r + 1) * 128]
                ps = ps_g.tile([128, DIM], F32, tag="gram")
                nc.tensor.matmul(
                    ps[:, 0:512], lhsT, xT[:, 0:512],
                    start=True, stop=False, skip_group_check=True,
                )
                nc.tensor.matmul(
                    ps[:, 512:640], lhsT, xT[:, 512:640],
                    start=True, stop=False, skip_group_check=True,
                )
                nc.tensor.matmul(
                    ps[:, 0:512], c_ones2[:], hrow1[:, 0:512],
                    start=False, stop=True, skip_group_check=True,
                )
                nc.tensor.matmul(
                    ps[:, 512:640], c_ones2[:], hrow1[:, 512:640],
                    start=False, stop=True, skip_group_check=True,
                )
                nc.scalar.activation(
                    z[:, r, :], ps[:], AF.Sqrt,
                    bias=bias_s[:, r : r + 1],
                    scale=neg2alpha,
                    accum_out=rowsum_s[:, r : r + 1],
                )
                if r == 1 and mid_cb is not None:
                    mid_cb()

        # ---- per-sample software pipeline, head 2 samples ahead ----
        xbgs = {0: in_dma(0), 1: in_dma(1), 2: in_dma(2)}
        heads = {0: emit_head(0, xbgs[0]), 1: emit_head(1, xbgs[0])}
        prev = None
        for n in range(B_CORE):
            g = n // 2
            if n + 2 < B_CORE:
                g2 = (n + 2) // 2
                heads[n + 2] = emit_head(n + 2, xbgs[g2])
                if n % 2 == 0 and g + 3 < NG:
                    xbgs[g + 3] = in_dma(g + 3)
            if prev is not None:
                tail_prep(prev)
            z = zp.tile([128, NCHUNK, DIM], F32, tag="z")
            mid = (lambda p=prev: tail_colmean(p)) if prev is not None else None
            head = heads.pop(n)
            emit_sample_c(head, z, mid)
            if prev is not None:
                tail_stt(prev)
            rs_tot = psamp.tile([128, 1], F32, tag="rs")
            nc.vector.tensor_reduce(
                rs_tot[:], head[3][:],
                axis=mybir.AxisListType.X, op=OP.add,
            )
            prev = {"n": n, "z": z, "rowsum": head[3], "rs": rs_tot}

        tail_prep(prev)
        tail_colmean(prev)
        tail_stt(prev)

    nc.compile()
    return nc


def _get_nc():
    global _cached_nc
    if _cached_nc is None:
        _cached_nc = build()
    return _cached_nc


def make_in_maps(x: np.ndarray, t: np.ndarray):
    alpha = float(np.exp(t.astype(np.float64))[0, 0])
    consts = np.zeros((128, 2), dtype=np.float32)
    consts[:, 0] = -2.0 * alpha
    consts[:, 1] = 2.0 * alpha
    ident = np.eye(128, dtype=np.float32)
    xs = x.reshape(N_CORES, B_CORE, DIM, M)
    return [
        {"x": np.ascontiguousarray(xs[c]), "consts": consts, "ident": ident}
        for c in range(N_CORES)
    ]


def kernel(x: np.ndarray, t: np.ndarray) -> np.ndarray:
    x = np.asarray(x, dtype=np.float32)
    t = np.asarray(t, dtype=np.float32)
    nc = _get_nc()
    res = run_bass_kernel_spmd(nc, make_in_maps(x, t), core_ids=list(range(N_CORES)))
    return np.concatenate([r["out"] for r in res.results], axis=0)
